# revision 1
# baseline (speedup 1.0000x reference)
"""Self-contained Trainium2 Bass kernel for nn_CerebroOriginalBlock (dense
transformer block with int8 fake-quant, temporal attention, MLP).

Sharding: pure data-parallel over batch (B=8 -> 1 batch element per core).
Per-tensor activation-quant scales are computed with AllReduce-max
collectives across the 8 cores. Weights are pre-fake-quantized on the host
(int8 values carried as bf16, exact) and matmuls run on the PE in bf16 with
f32 PSUM accumulation, which is bit-exact for int8-valued operands.

All activations flow feature-major ([d, token]); the input/output
token-major layout is handled by PE transposes.
"""
import sys
import types

sys.path.insert(0, '/opt/trn_rl_repo')

import numpy as np
import ml_dtypes

import concourse.bass as bass
import concourse.mybir as mybir
import concourse.tile as tile
from concourse.vector_clock import ScopedClock
from concourse.bass_utils import run_bass_kernel_spmd
from concourse.masks import make_identity

F32 = mybir.dt.float32
BF16 = mybir.dt.bfloat16
ALU = mybir.AluOpType
AF = mybir.ActivationFunctionType
AX = mybir.AxisListType

# ---------------------------------------------------------------- constants
B = 8
NTOK = 4096          # C*T tokens per batch element
D = 768
DQKV = 2304
DMLP = 3072
C, T, H, DH = 64, 64, 12, 64
KD = D // 128        # 6   k-tiles over D
KM = DMLP // 128     # 24  k-tiles over mlp dim
G = 8                # groups of 512 tokens
GS = 512
MQK = 12             # q+k m-tiles (1536/128)
MV0 = 12             # v section starts at m-tile 12
MD = D // 128        # 6
MMLP = DMLP // 128   # 24
MAGIC = float(np.float32(3 * 2 ** 22))   # RNE round-to-int magic
EPS = 1e-5
SCALE = 1.0 / 8.0    # 1/sqrt(dh)
NCORES = 8


# -------------------------------------------------- environment workarounds
def _patched_drain_and_barrier(self, tick_clock, wait_clock):
    # This walrus build rejects Drain instructions carrying >=2 sem waits.
    nop = self.nc.sync.nop(nofuse=True)
    wait_clock.add_sem_waits(nop.ins, ScopedClock({None: tick_clock.global_clock}))
    si = nop.ins.sync_info
    waits = list(si.on_wait) if si and si.on_wait else []
    if len(waits) > 1:
        del si.on_wait[1:]
        for w in waits[1:]:
            n2 = self.nc.sync.nop(nofuse=True)
            n2.ins.sync_info = mybir.SyncInfo(on_wait=[w], on_update=[])
    self.nc.sync.drain()
    self.nc.all_engine_barrier()
    popped = self.nc._tile_sem_poison_stack.pop()
    assert popped is self._sem_poison
    self.nc.clear_and_free_semaphores(list(self.sems.allocated().values()))
    self.nc.all_engine_barrier()


tile.TileContext._drain_and_barrier = _patched_drain_and_barrier


def _fix_multiwait(nc, limit=1):
    """Hoist excess sem-waits onto same-engine NOPs (walrus 1-wait limit)."""
    for fn in nc.m.functions:
        for bb in fn.blocks:
            new_list = []
            for inst in bb.instructions:
                si = inst.sync_info
                waits = list(si.on_wait) if si and si.on_wait else []
                if len(waits) > limit:
                    for j, w in enumerate(waits[:-limit]):
                        nop = mybir.InstNoOp(name=f"{inst.name}-hw{j}", ins=[], outs=[])
                        nop.engine = inst.engine
                        nop.sync_info = mybir.SyncInfo(on_wait=[w], on_update=[])
                        new_list.append(nop)
                    del si.on_wait[:-limit]
                new_list.append(inst)
            bb.instructions[:] = new_list


# ----------------------------------------------------------- host-side prep
def _fq_weight(w):
    """Exact mirror of reference fq() for a weight tensor, returning the int
    values (as float32) and the scale, all in float32 arithmetic."""
    w = np.asarray(w, np.float32)
    s = np.float32(max(np.abs(w).max(), np.float32(1e-8)) / np.float32(127.0))
    q = np.clip(np.round(w / s), -128.0, 127.0).astype(np.float32)
    return q, s


def _prep_host(inputs):
    """Build per-core in_maps from the full inputs."""
    x = np.asarray(inputs["x"], np.float32)            # [8, 4096, 768]
    qkv_q, s_qkv = _fq_weight(inputs["qkv_w"])         # [2304, 768]
    proj_q, s_proj = _fq_weight(inputs["proj_w"])      # [768, 768]
    fc1_q, s_fc1 = _fq_weight(inputs["fc1_w"])         # [3072, 768]
    fc2_q, s_fc2 = _fq_weight(inputs["fc2_w"])         # [768, 3072]

    bf = ml_dtypes.bfloat16
    qkvT = np.ascontiguousarray(qkv_q.T).astype(bf)    # [768, 2304]
    projT = np.ascontiguousarray(proj_q.T).astype(bf)  # [768, 768]
    fc1T = np.ascontiguousarray(fc1_q.T).astype(bf)    # [768, 3072]
    fc2T = np.ascontiguousarray(fc2_q.T).astype(bf)    # [3072, 768]

    def cols(v, n):   # [n*128] -> [128, n]  (col k = v[k*128:(k+1)*128])
        return np.ascontiguousarray(
            np.asarray(v, np.float32).reshape(n, 128).T)

    ln1 = np.concatenate([cols(inputs["ln1_g"], KD), cols(inputs["ln1_b"], KD)], 1)
    ln2 = np.concatenate([cols(inputs["ln2_g"], KD), cols(inputs["ln2_b"], KD)], 1)
    b_qkv = cols(inputs["qkv_b"], 18)
    b_proj = cols(inputs["proj_b"], 6)
    b_fc1 = cols(inputs["fc1_b"], 24)
    b_fc2 = cols(inputs["fc2_b"], 6)
    vbias = cols(np.asarray(inputs["qkv_b"], np.float32)[1536:], 6)  # v-section bias
    vbrow = np.zeros((128, 768), np.float32)
    vbrow[0, :] = np.asarray(inputs["qkv_b"], np.float32)[1536:]
    vbrow[64, :] = vbrow[0, :]
    ones2 = np.zeros((128, 256), np.float32)
    ones2[0, :] = 1.0; ones2[64, :] = 1.0

    ones_col = np.ones((1, 128), np.float32)
    onescol0 = np.zeros((128, 2), np.float32); onescol0[:, 0] = 1.0
    onescol1 = np.zeros((128, 2), np.float32); onescol1[:, 1] = 1.0
    sel0 = np.zeros((2, 128), np.float32); sel0[0, :] = 1.0 / 768.0
    sel1 = np.zeros((2, 128), np.float32); sel1[1, :] = 1.0 / 768.0
    bd = np.zeros((128, 128), np.float32)
    bd[:64, :64] = 1.0; bd[64:, 64:] = 1.0                  # blockdiag64
    wscales = np.tile(np.array([[s_qkv, s_proj, s_fc1, s_fc2]], np.float32), (128, 1))

    shared = {
        "qkvT": qkvT, "projT": projT, "fc1T": fc1T, "fc2T": fc2T,
        "ln1": ln1, "ln2": ln2,
        "b_qkv": b_qkv, "b_proj": b_proj, "b_fc1": b_fc1, "b_fc2": b_fc2,
        "vbias": vbias,
        "ones_col": ones_col,
        "onescol0": onescol0.astype(bf), "onescol1": onescol1.astype(bf),
        "sel0": sel0, "sel1": sel1, "bd": bd.astype(bf),
        "wscales": wscales,
        "epscol": np.full((128, 1), np.float32(EPS), np.float32),
        "negmag": np.full((128, 1), np.float32(-MAGIC), np.float32),
        "vbrow": vbrow.astype(bf), "ones2": ones2.astype(bf),
    }
    in_maps = []
    for c in range(NCORES):
        m = dict(shared)
        m["x"] = np.ascontiguousarray(x[c])
        in_maps.append(m)
    return in_maps, (s_qkv, s_proj, s_fc1, s_fc2)


# ------------------------------------------------------------------ builder
def build_nc(debug=False):
    nc = bass.Bass()
    RG = [list(range(NCORES))]

    x_p = nc.declare_dram_parameter("x", [NTOK, D], F32, isOutput=False)
    qkvT_p = nc.declare_dram_parameter("qkvT", [D, DQKV], BF16, isOutput=False)
    projT_p = nc.declare_dram_parameter("projT", [D, D], BF16, isOutput=False)
    fc1T_p = nc.declare_dram_parameter("fc1T", [D, DMLP], BF16, isOutput=False)
    fc2T_p = nc.declare_dram_parameter("fc2T", [DMLP, D], BF16, isOutput=False)
    ln1_p = nc.declare_dram_parameter("ln1", [128, 2 * KD], F32, isOutput=False)
    ln2_p = nc.declare_dram_parameter("ln2", [128, 2 * KD], F32, isOutput=False)
    bqkv_p = nc.declare_dram_parameter("b_qkv", [128, 18], F32, isOutput=False)
    bproj_p = nc.declare_dram_parameter("b_proj", [128, 6], F32, isOutput=False)
    bfc1_p = nc.declare_dram_parameter("b_fc1", [128, 24], F32, isOutput=False)
    bfc2_p = nc.declare_dram_parameter("b_fc2", [128, 6], F32, isOutput=False)
    vbias_p = nc.declare_dram_parameter("vbias", [128, 6], F32, isOutput=False)
    onescol_p = nc.declare_dram_parameter("ones_col", [1, 128], F32, isOutput=False)
    oc0_p = nc.declare_dram_parameter("onescol0", [128, 2], BF16, isOutput=False)
    oc1_p = nc.declare_dram_parameter("onescol1", [128, 2], BF16, isOutput=False)
    sel0_p = nc.declare_dram_parameter("sel0", [2, 128], F32, isOutput=False)
    sel1_p = nc.declare_dram_parameter("sel1", [2, 128], F32, isOutput=False)
    bd_p = nc.declare_dram_parameter("bd", [128, 128], BF16, isOutput=False)
    wsc_p = nc.declare_dram_parameter("wscales", [128, 4], F32, isOutput=False)
    eps_p = nc.declare_dram_parameter("epscol", [128, 1], F32, isOutput=False)
    nmg_p = nc.declare_dram_parameter("negmag", [128, 1], F32, isOutput=False)
    vbrow_p = nc.declare_dram_parameter("vbrow", [128, 768], BF16, isOutput=False)
    ones2_p = nc.declare_dram_parameter("ones2", [128, 256], BF16, isOutput=False)

    out_p = nc.declare_dram_parameter("out", [NTOK, D], F32, isOutput=True)
    dbg = {}
    if debug:
        for name, shape, dt in [
                ("dbg_h1q", [128, GS], BF16), ("dbg_qk", [128, GS], BF16),
                ("dbg_v", [128, D], BF16), ("dbg_o", [128, 2 * GS], BF16),
                ("dbg_x1", [128, GS], F32), ("dbg_h2q", [128, GS], BF16),
                ("dbg_gl", [128, GS], BF16), ("dbg_scl", [1, 4], F32)]:
            dbg[name] = nc.declare_dram_parameter(name, shape, dt, isOutput=True)

    xT_dram = nc.dram_tensor("xT_dram", [D, NTOK], F32)
    x1_dram = nc.dram_tensor("x1_dram", [D, NTOK], F32)
    g_dram = nc.dram_tensor("g_dram", [DMLP, NTOK], BF16)
    ccs = []
    for i in range(4):
        ci = nc.dram_tensor(f"cc{i}_in", [1, 1], F32)
        co = nc.dram_tensor(f"cc{i}_out", [1, 1], F32, addr_space="Shared")
        ccs.append((ci, co))

    with tile.TileContext(nc) as tc:
        from contextlib import ExitStack
        with ExitStack() as ctx:
            cpool = ctx.enter_context(tc.tile_pool(name="const", bufs=1))
            resp = ctx.enter_context(tc.tile_pool(name="res", bufs=1))
            sml = ctx.enter_context(tc.tile_pool(name="sml", bufs=1))
            scrA = ctx.enter_context(tc.tile_pool(name="scrA", bufs=1))
            scrB = ctx.enter_context(tc.tile_pool(name="scrB", bufs=2))
            mrp_pool = ctx.enter_context(tc.tile_pool(name="mr", bufs=2))

            def ld(pool, p, shape, dtype, tag):
                t = pool.tile(shape, dtype, tag=tag, name=tag)
                nc.sync.dma_start(t[:], p[:])
                return t

            ident = cpool.tile([128, 128], F32, tag="ident")
            make_identity(nc, ident[:])
            onescol = ld(cpool, onescol_p, [1, 128], F32, "onescol")
            oc0 = ld(cpool, oc0_p, [128, 2], BF16, "oc0")
            oc1 = ld(cpool, oc1_p, [128, 2], BF16, "oc1")
            sel0 = ld(cpool, sel0_p, [2, 128], F32, "sel0")
            sel1 = ld(cpool, sel1_p, [2, 128], F32, "sel1")
            bd = ld(cpool, bd_p, [128, 128], BF16, "bd")
            ln1 = ld(cpool, ln1_p, [128, 2 * KD], F32, "ln1")
            ln2 = ld(cpool, ln2_p, [128, 2 * KD], F32, "ln2")
            bqkv = ld(cpool, bqkv_p, [128, 18], F32, "bqkv")
            bproj = ld(cpool, bproj_p, [128, 6], F32, "bproj")
            bfc1 = ld(cpool, bfc1_p, [128, 24], F32, "bfc1")
            bfc2 = ld(cpool, bfc2_p, [128, 6], F32, "bfc2")
            vbias = ld(cpool, vbias_p, [128, 6], F32, "vbias")
            wsc = ld(cpool, wsc_p, [128, 4], F32, "wsc")
            epscol = ld(cpool, eps_p, [128, 1], F32, "epscol")
            negmag = ld(cpool, nmg_p, [128, 1], F32, "negmag")
            vbrow = ld(cpool, vbrow_p, [128, 768], BF16, "vbrow")
            ones2 = ld(cpool, ones2_p, [128, 256], BF16, "ones2")
            scl_dbg = cpool.tile([1, 4], F32, tag="scl_dbg")

            # residents: h (h1 then h2 reuse) and big2 (o_res, then fc1w, fc2w)
            h1 = resp.tile([128, KD * NTOK], BF16, tag="hbig", name="h1")
            o_res = resp.tile([128, 6 * NTOK], BF16, tag="big2", name="o_res")
            hmax = sml.tile([128, 1], F32, tag="hmax")
            omax = sml.tile([128, 1], F32, tag="omax")
            h2max = sml.tile([128, 1], F32, tag="h2max")
            gmax = sml.tile([128, 1], F32, tag="gmax")
            for t in (hmax, omax, h2max, gmax):
                nc.gpsimd.memset(t[:], 0.0)

            # ---------------- helpers
            def ln_stats_group(src, ps_stats, ps_rep):
                """src [128, KD*GS] f32 -> (m_chunk, r_chunk) [128, GS] f32."""
                xb = scrA.tile([128, KD * GS], BF16, tag="ln_xb", name="ln_xb")
                nc.vector.tensor_copy(xb[:], src[:])  # DVE 2x f32 copy
                xq = scrA.tile([128, KD * GS], BF16, tag="ln_xq", name="ln_xq")
                nc.scalar.activation(xq[:], src[:], AF.Square)
                st = ps_stats.tile([2, GS], F32, tag="ln_st", name="ln_st")
                for k in range(KD):
                    nc.tensor.matmul(st[:], oc0[:], xb[:, k * GS:(k + 1) * GS],
                                     start=(k == 0), stop=False)
                for k in range(KD):
                    nc.tensor.matmul(st[:], oc1[:], xq[:, k * GS:(k + 1) * GS],
                                     start=False, stop=(k == KD - 1))
                stsb = scrA.tile([2, GS], F32, tag="ln_stsb", name="ln_stsb")
                nc.scalar.copy(stsb[:], st[:])
                mrp = ps_rep.tile([128, GS], F32, tag="ln_mrp", name="ln_mrp")
                nc.tensor.matmul(mrp[:], sel0[:], stsb[:], start=True, stop=True)
                e2p = ps_rep.tile([128, GS], F32, tag="ln_e2p", name="ln_e2p")
                nc.tensor.matmul(e2p[:], sel1[:], stsb[:], start=True, stop=True)
                m_ch = mrp_pool.tile([128, GS], F32, tag="m_ch", name="m_ch")
                nc.scalar.copy(m_ch[:], mrp[:])
                v_ch = scrA.tile([128, GS], F32, tag="ln_v", name="ln_v")
                nc.scalar.activation(v_ch[:], mrp[:], AF.Square)   # m^2
                nc.vector.tensor_tensor(v_ch[:], e2p[:], v_ch[:], ALU.subtract)
                nc.scalar.activation(v_ch[:], v_ch[:], AF.Ln, bias=epscol[:])
                r_ch = mrp_pool.tile([128, GS], F32, tag="r_ch", name="r_ch")
                nc.scalar.activation(r_ch[:], v_ch[:], AF.Exp, scale=-0.5)
                return m_ch, r_ch

            def ln_maxh_chunk(src_ap, k, m_ch, r_ch, hdst_ap, gb, hmx):
                u = scrA.tile([128, GS], F32, tag="ln_u", name="ln_u")
                nc.vector.tensor_tensor(u[:], src_ap, m_ch[:], ALU.subtract)
                nc.vector.tensor_tensor(u[:], u[:], r_ch[:], ALU.mult)
                nc.scalar.activation(hdst_ap, u[:], AF.Identity,
                                     scale=gb[:, k:k + 1],
                                     bias=gb[:, KD + k:KD + k + 1])
                hm = scrB.tile([128, 1], F32, tag="ln_hm", name="ln_hm")
                nc.vector.tensor_reduce(hm[:], hdst_ap, AX.X, ALU.max,
                                        apply_absolute_value=True)
                nc.vector.tensor_tensor(hmx[:], hmx[:], hm[:], ALU.max)

            def scale_from_max(idx, col):
                ci, co = ccs[idx]
                with tc.tile_pool(name=f"ccps{idx}", bufs=1, space="PSUM") as pp:
                    tp = pp.tile([1, 128], F32, tag="cc_tp", name="cc_tp")
                    nc.tensor.transpose(tp[:], col[:], ident[:])
                    red = sml.tile([1, 1], F32, tag=f"cc_red{idx}")
                    nc.vector.tensor_reduce(red[:], tp[:], AX.X, ALU.max)
                    nc.sync.dma_start(ci[:], red[:])
                    nc.gpsimd.collective_compute(
                        "AllReduce", ALU.max, replica_groups=RG,
                        ins=[ci[:]], outs=[co[:]])
                    mx = sml.tile([1, 1], F32, tag=f"cc_mx{idx}")
                    nc.sync.dma_start(mx[:], co[:])
                    s = sml.tile([1, 1], F32, tag=f"cc_s{idx}")
                    nc.vector.tensor_scalar(s[:], mx[:], float(np.float32(1 / 127.0)),
                                            None, ALU.mult)
                    rcp = sml.tile([1, 1], F32, tag=f"cc_r{idx}")
                    nc.vector.reciprocal(rcp[:], mx[:])
                    is_ = sml.tile([1, 1], F32, tag=f"cc_is{idx}")
                    nc.vector.tensor_scalar(is_[:], rcp[:], 127.0, None, ALU.mult)
                    nc.vector.tensor_copy(scl_dbg[:, idx:idx + 1], s[:])
                return s, is_

            def rep128(val, tag):
                with tc.tile_pool(name=f"rp_{tag}", bufs=1, space="PSUM") as pp:
                    rp = pp.tile([128, 1], F32, tag="rep_ps", name="rep_ps")
                    nc.tensor.matmul(rp[:], onescol[:], val[:], start=True, stop=True)
                    t = sml.tile([128, 1], F32, tag=tag)
                    nc.scalar.copy(t[:], rp[:])
                return t

            def quant_chunk(big, slot, lo, is_rep, width=NTOK):
                sl = big[:, slot * width + lo: slot * width + lo + GS]
                t = scrA.tile([128, GS], F32, tag="q_t", name="q_t")
                nc.vector.tensor_scalar(t[:], sl, is_rep[:], MAGIC, ALU.mult, ALU.add)
                nc.scalar.activation(sl, t[:], AF.Identity, bias=negmag[:])

            # ============================ P1: load x, transpose, LN1, spill
            with (
                tc.tile_pool(name="xgp", bufs=2) as xgp,
                tc.tile_pool(name="p1ps", bufs=2, space="PSUM") as p1ps,
                tc.tile_pool(name="p1st", bufs=1, space="PSUM") as p1st,
                tc.tile_pool(name="p1rep", bufs=1, space="PSUM") as p1rep,
            ):
                for g in range(G):
                    xg = xgp.tile([128, KD * GS], F32, tag="xg", name="xg")
                    for tt in range(4):
                        xin = scrB.tile([128, D], F32, tag="xin", name="xin")
                        tok0 = (g * 4 + tt) * 128
                        nc.sync.dma_start(xin[:], x_p[tok0:tok0 + 128, :])
                        for k in range(KD):
                            xtp = p1ps.tile([128, 128], F32, tag="xtp", name="xtp")
                            nc.tensor.transpose(
                                xtp[:], xin[:, k * 128:(k + 1) * 128], ident[:])
                            nc.scalar.copy(
                                xg[:, k * GS + tt * 128: k * GS + tt * 128 + 128],
                                xtp[:])
                    for k in range(KD):
                        nc.sync.dma_start(
                            xT_dram[k * 128:(k + 1) * 128, g * GS:(g + 1) * GS],
                            xg[:, k * GS:(k + 1) * GS])
                    m_ch, r_ch = ln_stats_group(xg, p1st, p1rep)
                    for k in range(KD):
                        ln_maxh_chunk(xg[:, k * GS:(k + 1) * GS], k, m_ch, r_ch,
                                      h1[:, k * NTOK + g * GS: k * NTOK + (g + 1) * GS],
                                      ln1, hmax)

            # ========================= P2: quantize h1, qkv+v, attention
            with (
                tc.tile_pool(name="qkvg", bufs=1) as qvp,
                tc.tile_pool(name="wqkp", bufs=2) as wst,
                tc.tile_pool(name="wvp", bufs=1) as wvp,
            ):
                wv = wvp.tile([128, KD * 768], BF16, tag="wv", name="wv")
                nc.sync.dma_start(
                    wv[:].rearrange("p (k c) -> p k c", c=768),
                    qkvT_p[:, 1536:2304].rearrange("(k p) c -> p k c", p=128))
                # scale 1 (post-LN1 quant)
                s1, is1 = scale_from_max(0, hmax)
                is1r = rep128(is1, "is1r")
                s1r = rep128(s1, "s1r")
                dq_qkv = sml.tile([128, 1], F32, tag="dq_qkv")
                nc.vector.tensor_scalar(dq_qkv[:], s1r[:], wsc[:, 0:1], None, ALU.mult)
                qk_tiles = {}
                vg_tiles = {}
                for sg in range(4):
                    gs_pair = (2 * sg, 2 * sg + 1)
                    with (
                        tc.tile_pool(name=f"qps{sg}", bufs=3, space="PSUM") as qps,
                        tc.tile_pool(name=f"vps{sg}", bufs=1, space="PSUM") as vps,
                    ):
                        for g in gs_pair:
                            for k in range(KD):
                                quant_chunk(h1, k, g * GS, is1r)
                            if debug and g == 0:
                                nc.sync.dma_start(dbg["dbg_h1q"][:], h1[:, 0:GS])
                            qk = qvp.tile([128, MQK * GS], BF16,
                                          tag=f"qk{g % 2}", name=f"qk{g % 2}")
                            vg = qvp.tile([128, 4 * D], BF16,
                                          tag=f"vg{g % 2}", name=f"vg{g % 2}")
                            qk_tiles[g % 2] = qk
                            vg_tiles[g % 2] = vg
                            for m in range(MQK):
                                wqk = wst.tile([128, KD * 128], BF16, tag="wqk",
                                               name="wqk")
                                nc.sync.dma_start(
                                    wqk[:].rearrange("p (k c) -> p k c", c=128),
                                    qkvT_p[:, m * 128:(m + 1) * 128].rearrange(
                                        "(k p) c -> p k c", p=128))
                                ps = qps.tile([128, GS], F32, tag="qkv_ps",
                                              name="qkv_ps")
                                for k in range(KD):
                                    nc.tensor.matmul(
                                        ps[:],
                                        wqk[:, k * 128:(k + 1) * 128],
                                        h1[:, k * NTOK + g * GS:
                                           k * NTOK + (g + 1) * GS],
                                        start=(k == 0), stop=(k == KD - 1))
                                nc.vector.tensor_scalar(
                                    qk[:, m * GS:(m + 1) * GS], ps[:], dq_qkv[:],
                                    bqkv[:, m:m + 1], ALU.mult, ALU.add)
                            if debug and g == 0:
                                nc.sync.dma_start(dbg["dbg_qk"][:], qk[:, 0:GS])
                            for tt in range(4):
                                tok0 = (g * 4 + tt) * 128
                                vp = vps.tile([128, D], F32, tag="v_ps",
                                              name="v_ps")
                                for k in range(KD):
                                    nc.tensor.matmul(
                                        vp[:, 0:512],
                                        h1[:, k * NTOK + tok0: k * NTOK + tok0 + 128],
                                        wv[:, k * 768: k * 768 + 512],
                                        start=(k == 0), stop=(k == KD - 1))
                                for k in range(KD):
                                    nc.tensor.matmul(
                                        vp[:, 512:768],
                                        h1[:, k * NTOK + tok0: k * NTOK + tok0 + 128],
                                        wv[:, k * 768 + 512: (k + 1) * 768],
                                        start=(k == 0), stop=(k == KD - 1))
                                nc.vector.tensor_scalar(vg[:, tt * D: tt * D + 512],
                                                        vp[:, 0:512], dq_qkv[:],
                                                        None, ALU.mult)
                                nc.vector.tensor_scalar(vg[:, tt * D + 512:(tt + 1) * D],
                                                        vp[:, 512:768], dq_qkv[:],
                                                        None, ALU.mult)
                            if debug and g == 0:
                                nc.sync.dma_start(dbg["dbg_v"][:], vg[:, 0:D])
                    with tc.tile_pool(name=f"aps{sg}", bufs=1, space="PSUM") as aps:
                        for g in gs_pair:
                            qk = qk_tiles[g % 2]
                            vg = vg_tiles[g % 2]
                            for jj in range(3):
                                sc = {}
                                for hp in range(2):
                                    for cp in range(2):
                                        sc[hp, cp] = aps.tile(
                                            [128, GS], F32, tag=f"sc{hp}{cp}",
                                            name=f"sc{hp}{cp}")
                                for hp in range(2):
                                    for cp in range(2):
                                        for pj in range(2):
                                            for ci in range(4):
                                                h = 4 * jj + 2 * pj + hp
                                                cl = cp + 2 * ci
                                                s8 = pj * 4 + ci
                                                mk = 6 + h // 2
                                                mq = h // 2
                                                kb = qk[hp * 64:hp * 64 + 64,
                                                        mk * GS + cl * 64:
                                                        mk * GS + cl * 64 + 64]
                                                qb = qk[hp * 64:hp * 64 + 64,
                                                        mq * GS + cl * 64:
                                                        mq * GS + cl * 64 + 64]
                                                nc.tensor.matmul(
                                                    sc[hp, cp][cp * 64:cp * 64 + 64,
                                                               s8 * 64:s8 * 64 + 64],
                                                    kb, qb, start=True, stop=True,
                                                    tile_position=(hp * 64, cp * 64))
                                esb = {}
                                for hp in range(2):
                                    esb[hp] = scrA.tile([128, GS], BF16,
                                                        tag=f"esb{hp}",
                                                        name=f"esb{hp}")
                                    for cp in range(2):
                                        nc.scalar.activation(
                                            esb[hp][cp * 64:cp * 64 + 64, :],
                                            sc[hp, cp][cp * 64:cp * 64 + 64, :],
                                            AF.Exp, scale=SCALE)
                                psb = {}
                                for hp in range(2):
                                    srep = aps.tile([128, GS], F32,
                                                    tag=f"ob{hp}0",
                                                    name=f"srep{hp}")
                                    nc.tensor.matmul(srep[:], bd[:], esb[hp][:],
                                                     start=True, stop=True)
                                    inv = scrA.tile([128, GS], F32,
                                                    tag=f"ldiv{hp}",
                                                    name=f"inv{hp}")
                                    nc.scalar.activation(inv[:], srep[:], AF.Ln)
                                    nc.scalar.activation(inv[:], inv[:], AF.Exp,
                                                         scale=-1.0)
                                    psb[hp] = scrA.tile([128, GS], BF16,
                                                        tag=f"psb{hp}",
                                                        name=f"psb{hp}")
                                    nc.vector.tensor_tensor(psb[hp][:], esb[hp][:],
                                                            inv[:], ALU.mult)
                                ob = {}
                                for hp in range(2):
                                    for cp in range(2):
                                        ob[hp, cp] = aps.tile(
                                            [128, GS], F32, tag=f"ob{hp}{cp}",
                                            name=f"ob{hp}{cp}")
                                for hp in range(2):
                                    for cp in range(2):
                                        for pj in range(2):
                                            for ci in range(4):
                                                h = 4 * jj + 2 * pj + hp
                                                cl = cp + 2 * ci
                                                s8 = pj * 4 + ci
                                                tt = cl // 2
                                                vb = vg[cp * 64:cp * 64 + 64,
                                                        tt * D + h * 64:
                                                        tt * D + h * 64 + 64]
                                                pb = psb[hp][cp * 64:cp * 64 + 64,
                                                             s8 * 64:s8 * 64 + 64]
                                                nc.tensor.matmul(
                                                    ob[hp, cp][hp * 64:hp * 64 + 64,
                                                               s8 * 64:s8 * 64 + 64],
                                                    vb, pb, start=True, stop=True,
                                                    tile_position=(cp * 64, hp * 64))
                                j0 = 2 * jj
                                for hp in range(2):
                                    for cp in range(2):
                                        dst = o_res[hp * 64:hp * 64 + 64,
                                                    j0 * NTOK:(j0 + 2) * NTOK]
                                        dst = dst.rearrange(
                                            "p (pj a b) -> p pj a b", pj=2, b=64)
                                        dst = dst[:, :, g * 8 + cp::2, :][:, :, 0:4, :]
                                        src = ob[hp, cp][hp * 64:hp * 64 + 64, :]
                                        src = src.rearrange(
                                            "p (pj a b) -> p pj a b", pj=2, b=64)
                                        nc.scalar.activation(dst, src, AF.Identity)
            if debug:
                nc.sync.dma_start(dbg["dbg_o"][:], o_res[:, 0:2 * GS])
            for j in range(6):
                for g in range(G):
                    ot2 = scrA.tile([128, GS], F32, tag="q_t", name="q_t")
                    nc.vector.tensor_scalar(
                        ot2[:], o_res[:, j * NTOK + g * GS: j * NTOK + (g + 1) * GS],
                        vbias[:, j:j + 1], None, ALU.add)
                    om = scrB.tile([128, 1], F32, tag="om", name="om")
                    nc.vector.tensor_reduce(om[:], ot2[:], AX.X, ALU.max,
                                            apply_absolute_value=True)
                    nc.vector.tensor_tensor(omax[:], omax[:], om[:], ALU.max)

            # ============================ P3: o quant, proj + add1, LN2
            # omax must be computed on o + v_bias (bias deferred to the quant)
            s_o, is_o = scale_from_max(1, omax)
            is_or = rep128(is_o, "is_or")
            s_or = rep128(s_o, "s_or")
            dq_proj = sml.tile([128, 1], F32, tag="dq_proj")
            nc.vector.tensor_scalar(dq_proj[:], s_or[:], wsc[:, 1:2], None, ALU.mult)
            vbm = sml.tile([128, 6], F32, tag="vbm")
            nc.vector.tensor_scalar(vbm[:], vbias[:], is_or[:], MAGIC,
                                    ALU.mult, ALU.add)

            h2 = resp.tile([128, KD * NTOK], BF16, tag="hbig", name="h2")
            with (
                tc.tile_pool(name="projTp", bufs=1) as wq2,
                tc.tile_pool(name="x1gp", bufs=2) as x1gp,
                tc.tile_pool(name="p3ps", bufs=3, space="PSUM") as p3ps,
                tc.tile_pool(name="p3st", bufs=1, space="PSUM") as p3st,
                tc.tile_pool(name="p3rep", bufs=1, space="PSUM") as p3rep,
            ):
                projT = wq2.tile([128, KD * D], BF16, tag="projT", name="projT")
                nc.sync.dma_start(
                    projT[:].rearrange("p (k c) -> p k c", c=D),
                    projT_p[:].rearrange("(k p) c -> p k c", p=128))
                for g in range(G):
                    for j in range(6):
                        sl = o_res[:, j * NTOK + g * GS: j * NTOK + (g + 1) * GS]
                        qt = scrA.tile([128, GS], F32, tag="q_t", name="q_t")
                        nc.vector.tensor_scalar(qt[:], sl, is_or[:], vbm[:, j:j + 1],
                                                ALU.mult, ALU.add)
                        nc.scalar.activation(sl, qt[:], AF.Identity, bias=negmag[:])
                    x1g = x1gp.tile([128, KD * GS], F32, tag="x1g", name="x1g")
                    for m in range(MD):
                        ps = p3ps.tile([128, GS], F32, tag="pr_ps", name="pr_ps")
                        for k in range(KD):
                            nc.tensor.matmul(
                                ps[:],
                                projT[:, k * D + m * 128: k * D + (m + 1) * 128],
                                o_res[:, k * NTOK + g * GS: k * NTOK + (g + 1) * GS],
                                start=(k == 0), stop=(k == KD - 1))
                        pr = scrB.tile([128, GS], F32, tag="fA", name="pr_ev")
                        nc.scalar.activation(pr[:], ps[:], AF.Identity,
                                             bias=bproj[:, m:m + 1], scale=dq_proj[:])
                        xc = scrB.tile([128, GS], F32, tag="fB", name="xc")
                        nc.sync.dma_start(
                            xc[:], xT_dram[m * 128:(m + 1) * 128, g * GS:(g + 1) * GS])
                        nc.vector.tensor_tensor(x1g[:, m * GS:(m + 1) * GS],
                                                pr[:], xc[:], ALU.add)
                    for k in range(KD):
                        nc.sync.dma_start(
                            x1_dram[k * 128:(k + 1) * 128, g * GS:(g + 1) * GS],
                            x1g[:, k * GS:(k + 1) * GS])
                    if debug and g == 0:
                        nc.sync.dma_start(dbg["dbg_x1"][:], x1g[:, 0:GS])
                    m_ch, r_ch = ln_stats_group(x1g, p3st, p3rep)
                    for k in range(KD):
                        ln_maxh_chunk(x1g[:, k * GS:(k + 1) * GS], k, m_ch, r_ch,
                                      h2[:, k * NTOK + g * GS: k * NTOK + (g + 1) * GS],
                                      ln2, h2max)

            # ============================ P4: h2 quant, fc1 + gelu
            s2, is2 = scale_from_max(2, h2max)
            is2r = rep128(is2, "is2r")
            s2r = rep128(s2, "s2r")
            dq_fc1 = sml.tile([128, 1], F32, tag="dq_fc1")
            nc.vector.tensor_scalar(dq_fc1[:], s2r[:], wsc[:, 2:3], None, ALU.mult)

            fc1w = resp.tile([128, KD * DMLP], BF16, tag="big2", name="fc1w")
            nc.sync.dma_start(
                fc1w[:].rearrange("p (k c) -> p k c", c=DMLP),
                fc1T_p[:].rearrange("(k p) c -> p k c", p=128))
            with tc.tile_pool(name="p4ps", bufs=3, space="PSUM") as p4ps:
                for g in range(G):
                    for k in range(KD):
                        quant_chunk(h2, k, g * GS, is2r)
                    if debug and g == 0:
                        nc.sync.dma_start(dbg["dbg_h2q"][:], h2[:, 0:GS])
                    for m in range(MMLP):
                        ps = p4ps.tile([128, GS], F32, tag="f1_ps", name="f1_ps")
                        for k in range(KD):
                            nc.tensor.matmul(
                                ps[:],
                                fc1w[:, k * DMLP + m * 128: k * DMLP + (m + 1) * 128],
                                h2[:, k * NTOK + g * GS: k * NTOK + (g + 1) * GS],
                                start=(k == 0), stop=(k == KD - 1))
                        gl = scrB.tile([128, GS], BF16, tag="bA", name="gl")
                        nc.scalar.activation(gl[:], ps[:], AF.Gelu,
                                             bias=bfc1[:, m:m + 1], scale=dq_fc1[:])
                        nc.sync.dma_start(
                            g_dram[m * 128:(m + 1) * 128, g * GS:(g + 1) * GS], gl[:])
                        gm = scrB.tile([128, 1], F32, tag="gm", name="gm")
                        nc.vector.tensor_reduce(gm[:], gl[:], AX.X, ALU.max,
                                                apply_absolute_value=True)
                        nc.vector.tensor_tensor(gmax[:], gmax[:], gm[:], ALU.max)
                        if debug and g == 0 and m == 0:
                            nc.sync.dma_start(dbg["dbg_gl"][:], gl[:])

            # ============================ P5: g quant, fc2 + add2, out transpose
            s_g, is_g = scale_from_max(3, gmax)
            is_gr = rep128(is_g, "is_gr")
            s_gr = rep128(s_g, "s_gr")
            dq_fc2 = sml.tile([128, 1], F32, tag="dq_fc2")
            nc.vector.tensor_scalar(dq_fc2[:], s_gr[:], wsc[:, 3:4], None, ALU.mult)

            fc2w = resp.tile([128, KM * D], BF16, tag="big2", name="fc2w")
            nc.sync.dma_start(
                fc2w[:].rearrange("p (k c) -> p k c", c=D),
                fc2T_p[:].rearrange("(k p) c -> p k c", p=128))
            with (
                tc.tile_pool(name="gqp", bufs=2) as gqp,
                tc.tile_pool(name="p5ps", bufs=3, space="PSUM") as p5ps,
                tc.tile_pool(name="p5tp", bufs=2, space="PSUM") as p5tp,
            ):
                for g in range(G):
                    gq = gqp.tile([128, KM * GS], BF16, tag="gq", name="gq")
                    nc.sync.dma_start(
                        gq[:].rearrange("p (k c) -> p k c", c=GS),
                        g_dram[:, g * GS:(g + 1) * GS].rearrange(
                            "(k p) c -> p k c", p=128))
                    for k in range(KM):
                        quant_chunk(gq, k, 0, is_gr, width=GS)
                    for m in range(MD):
                        ps = p5ps.tile([128, GS], F32, tag="f2_ps", name="f2_ps")
                        for k in range(KM):
                            nc.tensor.matmul(
                                ps[:],
                                fc2w[:, k * D + m * 128: k * D + (m + 1) * 128],
                                gq[:, k * GS:(k + 1) * GS],
                                start=(k == 0), stop=(k == KM - 1))
                        ev = scrB.tile([128, GS], F32, tag="fA", name="f2ev")
                        nc.vector.tensor_scalar(ev[:], ps[:], dq_fc2[:],
                                                bfc2[:, m:m + 1], ALU.mult, ALU.add)
                        xc1 = scrB.tile([128, GS], F32, tag="fB", name="xc1")
                        nc.sync.dma_start(
                            xc1[:], x1_dram[m * 128:(m + 1) * 128, g * GS:(g + 1) * GS])
                        of = scrB.tile([128, GS], F32, tag="fC", name="ofm")
                        nc.vector.tensor_tensor(of[:], ev[:], xc1[:], ALU.add)
                        for i in range(4):
                            tp = p5tp.tile([128, 128], F32, tag="otp", name="otp")
                            nc.tensor.transpose(tp[:], of[:, i * 128:(i + 1) * 128],
                                                ident[:])
                            ot = scrA.tile([128, 128], F32, tag="ott", name="ott")
                            nc.vector.tensor_copy(ot[:], tp[:])
                            nc.sync.dma_start(
                                out_p[g * GS + i * 128: g * GS + (i + 1) * 128,
                                      m * 128:(m + 1) * 128], ot[:])
            if debug:
                nc.sync.dma_start(dbg["dbg_scl"][:], scl_dbg[:])

    _fix_multiwait(nc)
    nc.finalize()
    return nc


# ------------------------------------------------------------------ runtime
_CACHE = {}


def _get_nc(debug=False):
    key = bool(debug)
    if key not in _CACHE:
        _CACHE[key] = build_nc(debug=key)
    return _CACHE[key]


def _run(inputs, debug=False, **kw):
    in_maps, scales = _prep_host(inputs)
    nc = _get_nc(debug)
    res = run_bass_kernel_spmd(nc, in_maps, list(range(NCORES)), **kw)
    return res, scales


def kernel(**inputs):
    res, _ = _run(inputs, debug=False)
    out = np.stack([np.asarray(res.results[c]["out"], np.float32)
                    for c in range(NCORES)], 0)
    return out

